# revision 65
# baseline (speedup 1.0000x reference)
"""Causal single-head attention (B=8, T=2048, C=1024, HS=64) on 8 trn2 cores.

Strategy: data-parallel over batch B — one batch element per NeuronCore.

Wavefront pipeline (per core), over 4 load-stages of 512 rows each; q
groups load DESCENDING (q3 first) while k/v ascend, so the last-arriving
bytes feed only the smallest remaining attention work:
  1. SWDGE cast-loads [128,4,1024] fp32 DRAM -> bf16 SBUF natural, one
     stage ahead; weights/mask ride the HWDGE queue during the first
     SWDGE descriptor generation.
  2. PE-transposes via identity matmuls write bf16 PSUM ([128,1024] = one
     bank = two c-chunks); paired evacuations round-robin DVE(3):ACT(1)
     (GPSIMD cannot touch PSUM).
  3. Projections (W^T stationary, x^T moving) -> kt/qt [64,T] bf16; V is
     PE-transposed back to natural and merged with the mask column into
     vx [128,16,65] so the softmax denominator falls out of the A@V
     matmul.
  4. Attention j-block-groups (ic, jbg) become ready as soon as
     proj(q,ic) and proj(k/v,jbg) exist and drain through a FIFO spliced
     between transpose steps, so the PE's in-order stream never parks
     behind ACT's exp. A global software-pipeline emits each A@V matmul
     behind the next scores matmul. exp on ACT with the 1/sqrt(HS) scale
     fused (no max subtraction: scores ~ N(0,1)); diagonal causal mask
     via a multiplicative upper-tri bf16 tile on DVE.
  5. Normalize per i-chunk: PE-transpose out^T, reciprocal of the
     denominator column, scalar-mul, one output DMA. Normalizes are
     gated on chunk completion and the tail round-robins the last two
     chunks so their chains pipeline across engines.
"""

import numpy as np

import concourse.bass as bass
import concourse.mybir as mybir
import concourse.tile as tile
from concourse.masks import make_identity, make_upper_triangular

B, T, C, HS = 8, 2048, 1024, 64
P = 128
NT = T // P  # 16 t-tiles
NCB = C // P  # 8 c-chunks
TI = 512  # i-chunk width
NG = T // TI  # 4 groups / i-chunks
GG = TI // P  # 4 t-tiles per group

F32 = mybir.dt.float32
BF16 = mybir.dt.bfloat16
I32 = mybir.dt.int32


def split_excess_waits(nc):
    """walrus supports 1 sync-wait per instruction (2 on EventSemaphore);
    Tile's final drain can accumulate more. Hoist excess waits onto NoOp
    carriers inserted immediately before the overloaded instruction."""
    for blk in nc.m.functions[0].blocks:
        insts = blk.instructions
        i = 0
        while i < len(insts):
            inst = insts[i]
            si = inst.sync_info
            cap = 2 if isinstance(inst, mybir.InstEventSemaphore) else 1
            if si is not None and si.on_wait and len(si.on_wait) > cap:
                waits = list(si.on_wait)
                si.on_wait = waits[:cap]
                carriers = []
                for w in waits[cap:]:
                    n = mybir.InstNoOp(
                        name=nc.get_next_instruction_name(), ins=[], outs=[]
                    )
                    n.engine = inst.engine
                    n.sync_info = mybir.SyncInfo(on_wait=[w], on_update=[])
                    nc.register_instruction(n)
                    carriers.append(n)
                for j, n in enumerate(carriers):
                    insts.insert(i + j, n)
                i += len(carriers)
            i += 1


def make_consts(tc, singles, mask, wq, wk, wv):
    """Iteration-invariant constants: identity, causal mask, weights, mask.

    Emitted after the first data loads: identity/umask are built directly in
    bf16 on the Pool engine, the weights go through SWDGE cast DMAs queued
    behind the first data loads, and only the mask cast touches DVE (its
    HWDGE load completes early, so it never blocks the DVE queue).
    """
    nc = tc.nc
    # mask [T] int32 -> [128, NT] fp32. First on the HWDGE queue so the DVE
    # cast never parks the in-order DVE queue in front of the evacuations.
    mask_i = singles.tile([P, NT], I32)
    nc.sync.dma_start(out=mask_i[:], in_=mask.rearrange("(tb p) -> p tb", p=P))
    mask_f = singles.tile([P, NT], F32)
    nc.vector.tensor_copy(out=mask_f[:], in_=mask_i[:])

    ident = singles.tile([P, P], F32)
    make_identity(nc, ident[:])
    ident_bf = singles.tile([P, P], BF16)
    make_identity(nc, ident_bf[:])
    # umask[jj, ii] = 1 where ii >= jj else 0 (keep causal i >= j)
    umask_bf = singles.tile([P, P], BF16)
    make_upper_triangular(nc, umask_bf[:], val=1.0, diag=True)

    # weights [C, HS] fp32 -> bf16 chunks [128, cb, HS]. HWDGE (SP) loads
    # slot into the idle DMA window while the first data load's SWDGE
    # descriptors generate; the casts run on the still-idle ACT engine.
    w_sb = []
    for name, w in (("wq", wq), ("wk", wk), ("wv", wv)):
        w_f = singles.tile([P, NCB, HS], F32, tag=f"wf_{name}")
        nc.sync.dma_start(out=w_f[:], in_=w.rearrange("(cb c) h -> c cb h", c=P))
        t_ = singles.tile([P, NCB, HS], BF16, tag=f"w_{name}")
        nc.scalar.copy(out=t_[:], in_=w_f[:])
        w_sb.append(t_)

    return ident, ident_bf, umask_bf, w_sb, mask_f


class Pools:
    """Working tile pools + shared emission state, created once per kernel."""

    def __init__(self, tc, ctx):
        self.nat = ctx.enter_context(tc.tile_pool(name="nat", bufs=7))
        self.xt = ctx.enter_context(tc.tile_pool(name="xt", bufs=3))
        self.proj = ctx.enter_context(tc.tile_pool(name="proj", bufs=2))
        self.ex = ctx.enter_context(tc.tile_pool(name="ex", bufs=6))
        self.misc = ctx.enter_context(tc.tile_pool(name="misc", bufs=2))
        # bf16 PSUM ring for transpose outputs: [128,1024] bf16 = one full
        # bank holds TWO c-chunks; one evac copy moves both (2x DVE rate)
        self.ps_tr = ctx.enter_context(tc.tile_pool(name="ps_tr", bufs=2, space="PSUM"))
        self.ps_sc = ctx.enter_context(tc.tile_pool(name="ps_sc", bufs=2, space="PSUM"))
        self.ps_proj = ctx.enter_context(
            tc.tile_pool(name="ps_proj", bufs=1, space="PSUM")
        )
        self.ps_av = ctx.enter_context(tc.tile_pool(name="ps_av", bufs=3, space="PSUM"))
        self.evac = 0
        self.nats = {}


def emit_load(nc, pl, name, x, g):
    nat = pl.nat.tile([P, GG, C], BF16, tag="nat")
    nc.gpsimd.dma_start(
        out=nat[:],
        in_=x[g * TI : (g + 1) * TI, :].rearrange("(tt p) c -> p tt c", p=P),
    )
    pl.nats[(name, g)] = nat


def attention_body(tc, consts, pl, q, k, v, out, phase=4, skip_g0_loads=False):
    """Emit one iteration of the attention kernel (per-core shapes).

    phase: 1=loads only, 2=+transposes, 3=+projections/Vext, 4=full.
    Phases <4 write a dummy result to out so the kernel stays well-formed.

    Emission is step-interleaved: attention blocks of i-chunk g-1 are
    spliced between the transpose/projection steps of load-group g so the
    PE never sits behind ACT's exp in its in-order stream.
    """
    nc = tc.nc
    ident, ident_bf, umask_bf, w_sb, mask_f = consts
    # engine round-robin for PSUM->SBUF evacuations (ACT also runs exp;
    # Pool also runs SWDGE descriptor generation)
    # GPSIMD cannot access PSUM on real HW — evacuations are DVE/ACT only
    evac_engines = [nc.vector, nc.vector, nc.vector, nc.scalar]
    widx = {"q": 0, "k": 1, "v": 2}

    # persistent per-iteration tensors (rotate across iterations)
    kt = pl.proj.tile([HS, T], BF16, tag="kt")
    qt = pl.proj.tile([HS, T], BF16, tag="qt")
    vx = pl.proj.tile([P, NT, HS + 1], BF16, tag="vx")
    last = [None]

    def tp_gen(name, g):
        """Generator: 8 transpose-chunk steps + 1 projection step."""
        nat = pl.nats.pop((name, g))
        last[0] = nat
        if phase < 2:
            return
        xt = pl.xt.tile([P, NCB, GG, P], BF16, tag="xt")
        for cbp in range(NCB // 2):
            tp = pl.ps_tr.tile([P, 2 * GG * P], BF16, tag="tr")
            for half in range(2):
                cb = 2 * cbp + half
                for tt in range(GG):
                    nc.tensor.transpose(
                        tp[:, half * GG * P + tt * P : half * GG * P + (tt + 1) * P],
                        nat[:, tt, cb * P : (cb + 1) * P],
                        ident_bf[:],
                    )
            eng = evac_engines[pl.evac % len(evac_engines)]
            dst = xt[:, 2 * cbp : 2 * cbp + 2, :, :]
            src = tp[:].rearrange("p (a b c) -> p a b c", a=2, b=GG)
            if eng is nc.scalar:
                nc.scalar.copy(out=dst, in_=src)
            else:
                eng.tensor_copy(out=dst, in_=src)
            pl.evac += 1
            yield
        last[0] = xt
        if phase < 3:
            return
        # projection: [64, 512] = W^T @ x^T, accumulated over c-chunks
        pps = pl.ps_proj.tile([HS, TI], F32, tag="pp", name="pps")
        for cb in range(NCB):
            nc.tensor.matmul(
                pps[:],
                lhsT=w_sb[widx[name]][:, cb, :],
                rhs=xt[:, cb, :, :],
                start=(cb == 0),
                stop=(cb == NCB - 1),
            )
        if name == "k":
            nc.vector.tensor_copy(out=kt[:, g * TI : (g + 1) * TI], in_=pps[:])
        elif name == "q":
            nc.vector.tensor_copy(out=qt[:, g * TI : (g + 1) * TI], in_=pps[:])
        else:
            # V: back to natural [t, 64], scaled by mask, plus the
            # mask column as softmax-denominator accumulator
            vts = pl.misc.tile([HS, TI], BF16, tag="vts")
            nc.vector.tensor_copy(out=vts[:], in_=pps[:])
            vtp = pl.ps_tr.tile([P, 2 * GG * P], BF16, tag="tr")
            for tt in range(GG):
                nc.tensor.transpose(
                    vtp[:, tt * HS : (tt + 1) * HS],
                    vts[:, tt * P : (tt + 1) * P],
                    ident_bf[0:HS, 0:HS],
                )
            for tt in range(GG):
                tb = g * GG + tt
                nc.vector.tensor_scalar_mul(
                    out=vx[:, tb, 0:HS],
                    in0=vtp[:, tt * HS : (tt + 1) * HS],
                    scalar1=mask_f[:, tb : tb + 1],
                )
                nc.gpsimd.tensor_copy(
                    out=vx[:, tb, HS : HS + 1], in_=mask_f[:, tb : tb + 1]
                )
        yield

    # ---- wavefront attention: chunk ic's j-block-group jbg becomes ready
    # as soon as proj(q, ic) and proj(k/v, jbg) exist; groups drain through
    # a FIFO spliced between transpose steps. One global `pend` software-
    # pipelines every av matmul behind the next sc matmul.
    chunk_state = {}
    pend = [None]

    def chunk_st(ic):
        if ic not in chunk_state:
            chunk_state[ic] = {
                "av": pl.ps_av.tile([HS + 1, TI], F32, tag="av", name="av"),
                "n": 0,
                "emitted": 0,
                "total": GG * (ic + 1),
            }
        return chunk_state[ic]

    def flush_pend():
        if pend[0] is None:
            return
        ic, jb, ex, o, w = pend[0]
        pend[0] = None
        st = chunk_state[ic]
        nc.tensor.matmul(
            st["av"][:, o:],
            lhsT=vx[:, jb, :],
            rhs=ex[:, :w],
            start=(st["n"] == 0),
            stop=(st["n"] == st["total"] - 1),
        )
        st["n"] += 1

    def attn_group_gen(ic, jbg):
        st = chunk_st(ic)
        for jb in range(GG * jbg, GG * jbg + GG):
            o = max(0, jb * P - ic * TI)
            w = TI - o
            sc = pl.ps_sc.tile([P, TI], F32, tag="sc")
            nc.tensor.matmul(
                sc[:, :w],
                lhsT=kt[:, jb * P : (jb + 1) * P],
                rhs=qt[:, ic * TI + o : (ic + 1) * TI],
                start=True,
                stop=True,
            )
            flush_pend()
            ex = pl.ex.tile([P, TI], BF16, tag="ex")
            nc.scalar.activation(
                out=ex[:, :w],
                in_=sc[:, :w],
                func=mybir.ActivationFunctionType.Exp,
                scale=float(HS) ** -0.5,
            )
            if jbg == ic:
                # diagonal block: zero out j > i entries
                nc.vector.tensor_mul(ex[:, 0:P], ex[:, 0:P], umask_bf[:])
            pend[0] = (ic, jb, ex, o, w)
            st["emitted"] += 1
            yield

    def norm_gen(ic):
        # chunk ic's last block is either still pending (flush it) or was
        # already flushed by a later-queued group's sc step
        if pend[0] is not None and pend[0][0] == ic:
            flush_pend()
        av = chunk_state[ic]["av"]
        # normalize + emit
        oun = pl.misc.tile([HS + 1, TI], F32, tag="oun")
        nc.vector.tensor_copy(out=oun[:], in_=av[:])
        yield
        otp = pl.ps_sc.tile([P, TI], F32, tag="sc")
        for tt in range(GG):
            nc.tensor.transpose(
                otp[:, tt * (HS + 1) : (tt + 1) * (HS + 1)],
                oun[:, tt * P : (tt + 1) * P],
                ident[0 : HS + 1, 0 : HS + 1],
            )
        yield
        rden = pl.misc.tile([P, GG], F32, tag="rden")
        for tt in range(GG):
            nc.vector.reciprocal(
                out=rden[:, tt : tt + 1],
                in_=otp[:, tt * (HS + 1) + HS : (tt + 1) * (HS + 1)],
            )
        osb = pl.misc.tile([P, GG, HS], F32, tag="osb")
        for tt in range(GG):
            nc.vector.tensor_scalar_mul(
                out=osb[:, tt, :],
                in0=otp[:, tt * (HS + 1) : tt * (HS + 1) + HS],
                scalar1=rden[:, tt : tt + 1],
            )
        nc.sync.dma_start(
            out=out[ic * TI : (ic + 1) * TI, :].rearrange(
                "(tt p) h -> p tt h", p=P
            ),
            in_=osb[:],
        )
        yield

    from collections import deque

    fore_q = deque()
    norms_q = deque()  # (ic, gen) — gated until the chunk is fully emitted

    def step_norm():
        while norms_q:
            ic, gen = norms_q[0]
            st = chunk_state.get(ic)
            if st is None or st["emitted"] < st["total"]:
                return False
            try:
                next(gen)
                return True
            except StopIteration:
                norms_q.popleft()
        return False

    def pump():
        """Run one ready attention step (norms preferred once safe)."""
        if step_norm():
            return True
        while fore_q:
            try:
                next(fore_q[0])
                return True
            except StopIteration:
                fore_q.popleft()
        return False

    def run_tp(name, g):
        for _ in tp_gen(name, g):
            pump()

    # q load-groups descend (q3 first) so the last-arriving loads feed only
    # the smallest remaining attention work (tail = groups (0,0) and (3,3))
    qorder = [NG - 1 - s for s in range(NG)]
    if not skip_g0_loads:
        for name, x, g0 in (("q", q, qorder[0]), ("k", k, 0), ("v", v, 0)):
            emit_load(nc, pl, name, x, g0)
    for s in range(NG):
        if s + 1 < NG:
            for name, x, gn in (("q", q, qorder[s + 1]), ("k", k, s + 1), ("v", v, s + 1)):
                emit_load(nc, pl, name, x, gn)
        def add_norm(ic):
            norms_q.append((ic, norm_gen(ic)))

        run_tp("q", qorder[s])
        if phase >= 4:
            # groups enabled by this stage's q projection (k/v from earlier)
            qg = qorder[s]
            hi = min(qg, s - 1)
            if hi >= 0:
                for jbg in range(hi, -1, -1):  # diag-first
                    fore_q.append(attn_group_gen(qg, jbg))
                if hi == qg:
                    add_norm(qg)
        run_tp("k", s)
        run_tp("v", s)
        if phase >= 4:
            # groups enabled by this stage's k/v projections (jbg = s)
            for ic in range(NG - 1, max(s, NG - 1 - s) - 1, -1):
                fore_q.append(attn_group_gen(ic, s))
                if s == ic:
                    add_norm(ic)

    # drain: round-robin the leftover block groups so the tail chunks
    # pipeline across engines; norms fire as their chunks complete
    while fore_q or norms_q:
        pn = step_norm()
        pb = False
        if fore_q:
            try:
                next(fore_q[0])
                pb = True
                if len(fore_q) > 1:
                    fore_q.rotate(-1)
            except StopIteration:
                fore_q.popleft()
                pb = True
        if not (pn or pb) and not fore_q:
            assert not norms_q, "norm queue stuck with incomplete chunk"

    if phase < 4:
        dummy = pl.misc.tile([P, HS], F32, tag="dummy")
        if phase < 2:
            nc.vector.tensor_copy(out=dummy[:], in_=last[0][:, 0, 0:HS])
        elif phase < 3:
            nc.vector.tensor_copy(out=dummy[:], in_=last[0][:, 0, 0, 0:HS])
        else:
            nc.vector.tensor_copy(out=dummy[:], in_=vx[:, 0, 0:HS])
        nc.sync.dma_start(out=out[0:P, :], in_=dummy[:])


def build_nc(n_iters: int = 1, phase: int = 4):
    from contextlib import ExitStack

    # 64KB SWDGE ring (4096 descriptors) so up to 8 in-flight 512-descriptor
    # cast loads never block the Pool engine on descriptor-ring space.
    nc = bass.Bass(
        trn_type="TRN2", num_devices=B, dynamic_dma_scratch_size=65536
    )
    q = nc.declare_dram_parameter("q_vec", [T, C], F32, isOutput=False)
    k = nc.declare_dram_parameter("k_vec", [T, C], F32, isOutput=False)
    v = nc.declare_dram_parameter("v_vec", [T, C], F32, isOutput=False)
    mask = nc.declare_dram_parameter("mask", [T], I32, isOutput=False)
    wq = nc.declare_dram_parameter("Wq", [C, HS], F32, isOutput=False)
    wk = nc.declare_dram_parameter("Wk", [C, HS], F32, isOutput=False)
    wv = nc.declare_dram_parameter("Wv", [C, HS], F32, isOutput=False)
    out = nc.declare_dram_parameter("out", [T, HS], F32, isOutput=True)

    with tile.TileContext(nc) as tc:
        with ExitStack() as ctx:
            singles = ctx.enter_context(tc.tile_pool(name="singles", bufs=1))
            pl = Pools(tc, ctx)
            # the first data loads go ahead of the (HWDGE-issued) consts so
            # the Pool engine starts SWDGE descriptor generation immediately
            # (q loads descend: group NG-1 first — see attention_body)
            for name, x, g0 in (("q", q, NG - 1), ("k", k, 0), ("v", v, 0)):
                emit_load(nc, pl, name, x.ap(), g0)
            consts = make_consts(tc, singles, mask.ap(), wq.ap(), wk.ap(), wv.ap())
            for it in range(n_iters):
                attention_body(
                    tc,
                    consts,
                    pl,
                    q.ap(),
                    k.ap(),
                    v.ap(),
                    out.ap(),
                    phase=phase,
                    skip_g0_loads=(it == 0),
                )

    split_excess_waits(nc)
    return nc


# ---------------------------------------------------------------------------
# SPMD runner (compile once, execute via PJRT on the 8 axon cores)
# ---------------------------------------------------------------------------
class _Runner:
    def __init__(self, nc, n_cores=B):
        import jax
        from jax.sharding import Mesh, PartitionSpec
        from jax.experimental.shard_map import shard_map
        from concourse.bass2jax import (
            _bass_exec_p,
            install_neuronx_cc_hook,
            partition_id_tensor,
        )

        install_neuronx_cc_hook()
        self.jax = jax
        self.n_cores = n_cores
        partition_name = (
            nc.partition_id_tensor.name if nc.partition_id_tensor else None
        )

        in_names, out_names, out_avals, zero_outs = [], [], [], []
        for alloc in nc.m.functions[0].allocations:
            if not isinstance(alloc, mybir.MemoryLocationSet):
                continue
            name = alloc.memorylocations[0].name
            if alloc.kind == "ExternalInput":
                if name != partition_name:
                    in_names.append(name)
            elif alloc.kind == "ExternalOutput":
                out_names.append(name)
                shape = tuple(alloc.tensor_shape)
                dtype = mybir.dt.np(alloc.dtype)
                out_avals.append(jax.core.ShapedArray(shape, dtype))
                zero_outs.append(np.zeros(shape, dtype))
        self.in_names = list(in_names)
        self.out_names = out_names
        self.out_avals = out_avals
        self.zero_outs = zero_outs
        n_params = len(in_names)
        self.n_params = n_params

        all_in_names = list(in_names) + list(out_names)
        if partition_name is not None:
            all_in_names.append(partition_name)

        def _body(*args):
            operands = list(args)
            if partition_name is not None:
                operands.append(partition_id_tensor())
            outs = _bass_exec_p.bind(
                *operands,
                out_avals=tuple(out_avals),
                in_names=tuple(all_in_names),
                out_names=tuple(out_names),
                lowering_input_output_aliases=(),
                sim_require_finite=True,
                sim_require_nnan=True,
                nc=nc,
            )
            return tuple(outs)

        devices = jax.devices()[:n_cores]
        mesh = Mesh(np.asarray(devices), ("core",))
        n_outs = len(out_names)
        self.fn = jax.jit(
            shard_map(
                _body,
                mesh=mesh,
                in_specs=(PartitionSpec("core"),) * (n_params + n_outs),
                out_specs=(PartitionSpec("core"),) * n_outs,
                check_rep=False,
            ),
            keep_unused=True,
        )

    def prepare(self, in_maps):
        n = self.n_cores
        per_core = [[np.asarray(m[nm]) for nm in self.in_names] for m in in_maps]
        concat_in = [
            np.concatenate([per_core[c][i] for c in range(n)], axis=0)
            for i in range(self.n_params)
        ]
        concat_zeros = [
            np.zeros((n * z.shape[0], *z.shape[1:]), z.dtype) for z in self.zero_outs
        ]
        self.args = [self.jax.device_put(a) for a in concat_in + concat_zeros]
        return self

    def run(self):
        outs = self.fn(*self.args)
        self.jax.block_until_ready(outs)
        return outs

    def results(self, outs):
        n = self.n_cores
        return [
            {
                nm: np.asarray(outs[i]).reshape(n, *self.out_avals[i].shape)[c]
                for i, nm in enumerate(self.out_names)
            }
            for c in range(n)
        ]


_CACHED = {}


def _get_runner(n_iters: int = 1, phase: int = 4):
    key = (n_iters, phase)
    if key not in _CACHED:
        _CACHED[key] = _Runner(build_nc(n_iters, phase))
    return _CACHED[key]


def kernel(q_vec, k_vec, v_vec, mask, Wq, Wk, Wv):
    q_vec = np.ascontiguousarray(np.asarray(q_vec, dtype=np.float32))
    k_vec = np.ascontiguousarray(np.asarray(k_vec, dtype=np.float32))
    v_vec = np.ascontiguousarray(np.asarray(v_vec, dtype=np.float32))
    mask = np.ascontiguousarray(np.asarray(mask, dtype=np.int32))
    Wq = np.ascontiguousarray(np.asarray(Wq, dtype=np.float32))
    Wk = np.ascontiguousarray(np.asarray(Wk, dtype=np.float32))
    Wv = np.ascontiguousarray(np.asarray(Wv, dtype=np.float32))

    r = _get_runner()
    in_maps = [
        {
            "q_vec": q_vec[b],
            "k_vec": k_vec[b],
            "v_vec": v_vec[b],
            "mask": mask[b],
            "Wq": Wq,
            "Wk": Wk,
            "Wv": Wv,
        }
        for b in range(B)
    ]
    r.prepare(in_maps)
    res = r.results(r.run())
    return np.stack([res[b]["out"] for b in range(B)], axis=0)


# revision 73
# speedup vs baseline: 1.0000x; 1.0000x over previous
"""Causal single-head attention (B=8, T=2048, C=1024, HS=64) on 8 trn2 cores.

Strategy: data-parallel over batch B — one batch element per NeuronCore.

Wavefront pipeline (per core), over 4 load-stages of 512 rows each; q
groups load DESCENDING (q3 first) while k/v ascend, so the last-arriving
bytes feed only the smallest remaining attention work:
  1. SWDGE cast-loads [128,4,1024] fp32 DRAM -> bf16 SBUF natural, one
     stage ahead; weights/mask ride the HWDGE queue during the first
     SWDGE descriptor generation.
  2. PE-transposes via identity matmuls write bf16 PSUM ([128,1024] = one
     bank = two c-chunks); paired evacuations round-robin DVE(3):ACT(1)
     (GPSIMD cannot touch PSUM).
  3. Projections (W^T stationary, x^T moving) -> kt/qt [64,T] bf16; V is
     PE-transposed back to natural and merged with the mask column into
     vx [128,16,65] so the softmax denominator falls out of the A@V
     matmul.
  4. Attention j-block-groups (ic, jbg) become ready as soon as
     proj(q,ic) and proj(k/v,jbg) exist and drain through a FIFO spliced
     between transpose steps, so the PE's in-order stream never parks
     behind ACT's exp. A global software-pipeline emits each A@V matmul
     behind the next scores matmul. exp on ACT with the 1/sqrt(HS) scale
     fused (no max subtraction: scores ~ N(0,1)); diagonal causal mask
     via a multiplicative upper-tri bf16 tile on DVE.
  5. Normalize per i-chunk: PE-transpose out^T, reciprocal of the
     denominator column, scalar-mul, one output DMA. Normalizes are
     gated on chunk completion and the tail round-robins the last two
     chunks so their chains pipeline across engines.
"""

import numpy as np

import concourse.bass as bass
import concourse.mybir as mybir
import concourse.tile as tile
from concourse.masks import make_identity, make_upper_triangular

B, T, C, HS = 8, 2048, 1024, 64
P = 128
NT = T // P  # 16 t-tiles
NCB = C // P  # 8 c-chunks
TI = 512  # i-chunk width
NG = T // TI  # 4 groups / i-chunks
GG = TI // P  # 4 t-tiles per group

F32 = mybir.dt.float32
BF16 = mybir.dt.bfloat16
I32 = mybir.dt.int32


def split_excess_waits(nc):
    """walrus supports 1 sync-wait per instruction (2 on EventSemaphore);
    Tile's final drain can accumulate more. Hoist excess waits onto NoOp
    carriers inserted immediately before the overloaded instruction."""
    for blk in nc.m.functions[0].blocks:
        insts = blk.instructions
        i = 0
        while i < len(insts):
            inst = insts[i]
            si = inst.sync_info
            cap = 2 if isinstance(inst, mybir.InstEventSemaphore) else 1
            if si is not None and si.on_wait and len(si.on_wait) > cap:
                waits = list(si.on_wait)
                si.on_wait = waits[:cap]
                carriers = []
                for w in waits[cap:]:
                    n = mybir.InstNoOp(
                        name=nc.get_next_instruction_name(), ins=[], outs=[]
                    )
                    n.engine = inst.engine
                    n.sync_info = mybir.SyncInfo(on_wait=[w], on_update=[])
                    nc.register_instruction(n)
                    carriers.append(n)
                for j, n in enumerate(carriers):
                    insts.insert(i + j, n)
                i += len(carriers)
            i += 1


def make_consts(tc, singles, mask, wq, wk, wv):
    """Iteration-invariant constants: identity, causal mask, weights, mask.

    Emitted after the first data loads: identity/umask are built directly in
    bf16 on the Pool engine, the weights go through SWDGE cast DMAs queued
    behind the first data loads, and only the mask cast touches DVE (its
    HWDGE load completes early, so it never blocks the DVE queue).
    """
    nc = tc.nc
    # mask [T] int32 -> [128, NT] fp32. First on the HWDGE queue so the DVE
    # cast never parks the in-order DVE queue in front of the evacuations.
    mask_i = singles.tile([P, NT], I32)
    nc.sync.dma_start(out=mask_i[:], in_=mask.rearrange("(tb p) -> p tb", p=P))
    mask_f = singles.tile([P, NT], F32)
    nc.vector.tensor_copy(out=mask_f[:], in_=mask_i[:])

    ident = singles.tile([P, P], F32)
    make_identity(nc, ident[:])
    ident_bf = singles.tile([P, P], BF16)
    make_identity(nc, ident_bf[:])
    # umask[jj, ii] = 1 where ii >= jj else 0 (keep causal i >= j)
    umask_bf = singles.tile([P, P], BF16)
    make_upper_triangular(nc, umask_bf[:], val=1.0, diag=True)

    # weights [C, HS] fp32 -> bf16 chunks [128, cb, HS]. HWDGE (SP) loads
    # slot into the idle DMA window while the first data load's SWDGE
    # descriptors generate; the casts run on the still-idle ACT engine.
    w_sb = []
    for name, w in (("wq", wq), ("wk", wk), ("wv", wv)):
        w_f = singles.tile([P, NCB, HS], F32, tag=f"wf_{name}")
        nc.sync.dma_start(out=w_f[:], in_=w.rearrange("(cb c) h -> c cb h", c=P))
        t_ = singles.tile([P, NCB, HS], BF16, tag=f"w_{name}")
        nc.scalar.copy(out=t_[:], in_=w_f[:])
        w_sb.append(t_)

    return ident, ident_bf, umask_bf, w_sb, mask_f


class Pools:
    """Working tile pools + shared emission state, created once per kernel."""

    def __init__(self, tc, ctx):
        self.nat = ctx.enter_context(tc.tile_pool(name="nat", bufs=7))
        self.xt = ctx.enter_context(tc.tile_pool(name="xt", bufs=3))
        self.proj = ctx.enter_context(tc.tile_pool(name="proj", bufs=2))
        self.ex = ctx.enter_context(tc.tile_pool(name="ex", bufs=6))
        self.misc = ctx.enter_context(tc.tile_pool(name="misc", bufs=2))
        # bf16 PSUM ring for transpose outputs: [128,1024] bf16 = one full
        # bank holds TWO c-chunks; one evac copy moves both (2x DVE rate)
        self.ps_tr = ctx.enter_context(tc.tile_pool(name="ps_tr", bufs=2, space="PSUM"))
        self.ps_sc = ctx.enter_context(tc.tile_pool(name="ps_sc", bufs=2, space="PSUM"))
        self.ps_proj = ctx.enter_context(
            tc.tile_pool(name="ps_proj", bufs=1, space="PSUM")
        )
        self.ps_av = ctx.enter_context(tc.tile_pool(name="ps_av", bufs=3, space="PSUM"))
        # dedicated ring for the split first-group tiles so their one-shot
        # allocations never pin slots in the steady-state `nat` ring
        self.natt = ctx.enter_context(tc.tile_pool(name="natt", bufs=4))
        self.evac = 0
        self.nats = {}


def emit_load(nc, pl, name, x, g):
    nat = pl.nat.tile([P, GG, C], BF16, tag="nat")
    nc.gpsimd.dma_start(
        out=nat[:],
        in_=x[g * TI : (g + 1) * TI, :].rearrange("(tt p) c -> p tt c", p=P),
    )
    pl.nats[(name, g)] = nat


def emit_load_split(nc, pl, name, x, g):
    """Per-tile 0.5MB loads for the very first group, so the first transpose
    only waits for the first tile instead of the whole 2MB group."""
    tiles = []
    for tt in range(GG):
        natt = pl.natt.tile([P, C], BF16, tag="natt", name="natt")
        r0 = g * TI + tt * P
        nc.gpsimd.dma_start(out=natt[:], in_=x[r0 : r0 + P, :])
        tiles.append(natt)
    pl.nats[(name, g)] = tiles


def attention_body(tc, consts, pl, q, k, v, out, phase=4, skip_g0_loads=False):
    """Emit one iteration of the attention kernel (per-core shapes).

    phase: 1=loads only, 2=+transposes, 3=+projections/Vext, 4=full.
    Phases <4 write a dummy result to out so the kernel stays well-formed.

    Emission is step-interleaved: attention blocks of i-chunk g-1 are
    spliced between the transpose/projection steps of load-group g so the
    PE never sits behind ACT's exp in its in-order stream.
    """
    nc = tc.nc
    ident, ident_bf, umask_bf, w_sb, mask_f = consts
    # engine round-robin for PSUM->SBUF evacuations (ACT also runs exp;
    # Pool also runs SWDGE descriptor generation)
    # GPSIMD cannot access PSUM on real HW — evacuations are DVE/ACT only
    evac_engines = [nc.vector, nc.vector, nc.vector, nc.scalar]
    widx = {"q": 0, "k": 1, "v": 2}

    # persistent per-iteration tensors (rotate across iterations)
    kt = pl.proj.tile([HS, T], BF16, tag="kt")
    qt = pl.proj.tile([HS, T], BF16, tag="qt")
    vx = pl.proj.tile([P, NT, HS + 1], BF16, tag="vx")
    last = [None]

    def tp_gen(name, g):
        """Generator: 8 transpose-chunk steps + 1 projection step."""
        nat = pl.nats.pop((name, g))
        last[0] = nat
        if phase < 2:
            return
        split = isinstance(nat, list)  # per-tile loads (first q group only)
        xt = pl.xt.tile([P, NCB, GG, P], BF16, tag="xt")
        if split:
            # tt-major: all 8 c-chunks of one t-tile fill one PSUM bank, so
            # each step consumes a single 0.5MB load
            for tt in range(GG):
                tp = pl.ps_tr.tile([P, 2 * GG * P], BF16, tag="tr")
                for cb in range(NCB):
                    nc.tensor.transpose(
                        tp[:, cb * P : (cb + 1) * P],
                        nat[tt][:, cb * P : (cb + 1) * P],
                        ident_bf[:],
                    )
                eng = evac_engines[pl.evac % len(evac_engines)]
                dst = xt[:, :, tt, :]
                src = tp[:].rearrange("p (cb t) -> p cb t", cb=NCB)
                if eng is nc.scalar:
                    nc.scalar.copy(out=dst, in_=src)
                else:
                    eng.tensor_copy(out=dst, in_=src)
                pl.evac += 1
                yield
            last[0] = xt
            if phase < 3:
                return
            pps = pl.ps_proj.tile([HS, TI], F32, tag="pp", name="pps")
            for cb in range(NCB):
                nc.tensor.matmul(
                    pps[:],
                    lhsT=w_sb[widx[name]][:, cb, :],
                    rhs=xt[:, cb, :, :],
                    start=(cb == 0),
                    stop=(cb == NCB - 1),
                )
            nc.vector.tensor_copy(out=qt[:, g * TI : (g + 1) * TI], in_=pps[:])
            yield
            return
        for cbp in range(NCB // 2):
            tp = pl.ps_tr.tile([P, 2 * GG * P], BF16, tag="tr")
            for half in range(2):
                cb = 2 * cbp + half
                for tt in range(GG):
                    nc.tensor.transpose(
                        tp[:, half * GG * P + tt * P : half * GG * P + (tt + 1) * P],
                        nat[:, tt, cb * P : (cb + 1) * P],
                        ident_bf[:],
                    )
            eng = evac_engines[pl.evac % len(evac_engines)]
            dst = xt[:, 2 * cbp : 2 * cbp + 2, :, :]
            src = tp[:].rearrange("p (a b c) -> p a b c", a=2, b=GG)
            if eng is nc.scalar:
                nc.scalar.copy(out=dst, in_=src)
            else:
                eng.tensor_copy(out=dst, in_=src)
            pl.evac += 1
            yield
        last[0] = xt
        if phase < 3:
            return
        # projection: [64, 512] = W^T @ x^T, accumulated over c-chunks
        pps = pl.ps_proj.tile([HS, TI], F32, tag="pp", name="pps")
        for cb in range(NCB):
            nc.tensor.matmul(
                pps[:],
                lhsT=w_sb[widx[name]][:, cb, :],
                rhs=xt[:, cb, :, :],
                start=(cb == 0),
                stop=(cb == NCB - 1),
            )
        if name == "k":
            nc.vector.tensor_copy(out=kt[:, g * TI : (g + 1) * TI], in_=pps[:])
        elif name == "q":
            nc.vector.tensor_copy(out=qt[:, g * TI : (g + 1) * TI], in_=pps[:])
        else:
            # V: back to natural [t, 64], scaled by mask, plus the
            # mask column as softmax-denominator accumulator
            vts = pl.misc.tile([HS, TI], BF16, tag="vts")
            nc.vector.tensor_copy(out=vts[:], in_=pps[:])
            vtp = pl.ps_tr.tile([P, 2 * GG * P], BF16, tag="tr")
            for tt in range(GG):
                nc.tensor.transpose(
                    vtp[:, tt * HS : (tt + 1) * HS],
                    vts[:, tt * P : (tt + 1) * P],
                    ident_bf[0:HS, 0:HS],
                )
            for tt in range(GG):
                tb = g * GG + tt
                nc.vector.tensor_scalar_mul(
                    out=vx[:, tb, 0:HS],
                    in0=vtp[:, tt * HS : (tt + 1) * HS],
                    scalar1=mask_f[:, tb : tb + 1],
                )
                nc.gpsimd.tensor_copy(
                    out=vx[:, tb, HS : HS + 1], in_=mask_f[:, tb : tb + 1]
                )
        yield

    # ---- wavefront attention: chunk ic's j-block-group jbg becomes ready
    # as soon as proj(q, ic) and proj(k/v, jbg) exist; groups drain through
    # a FIFO spliced between transpose steps. One global `pend` software-
    # pipelines every av matmul behind the next sc matmul.
    chunk_state = {}
    pend = [None]

    def chunk_st(ic):
        if ic not in chunk_state:
            chunk_state[ic] = {
                "av": pl.ps_av.tile([HS + 1, TI], F32, tag="av", name="av"),
                "n": 0,
                "emitted": 0,
                "total": GG * (ic + 1),
            }
        return chunk_state[ic]

    def flush_pend():
        if pend[0] is None:
            return
        ic, jb, ex, o, w = pend[0]
        pend[0] = None
        st = chunk_state[ic]
        nc.tensor.matmul(
            st["av"][:, o:],
            lhsT=vx[:, jb, :],
            rhs=ex[:, :w],
            start=(st["n"] == 0),
            stop=(st["n"] == st["total"] - 1),
        )
        st["n"] += 1

    def attn_group_gen(ic, jbg):
        st = chunk_st(ic)
        for jb in range(GG * jbg, GG * jbg + GG):
            o = max(0, jb * P - ic * TI)
            w = TI - o
            sc = pl.ps_sc.tile([P, TI], F32, tag="sc")
            nc.tensor.matmul(
                sc[:, :w],
                lhsT=kt[:, jb * P : (jb + 1) * P],
                rhs=qt[:, ic * TI + o : (ic + 1) * TI],
                start=True,
                stop=True,
            )
            flush_pend()
            ex = pl.ex.tile([P, TI], BF16, tag="ex")
            nc.scalar.activation(
                out=ex[:, :w],
                in_=sc[:, :w],
                func=mybir.ActivationFunctionType.Exp,
                scale=float(HS) ** -0.5,
            )
            if jbg == ic:
                # diagonal block: zero out j > i entries
                nc.vector.tensor_mul(ex[:, 0:P], ex[:, 0:P], umask_bf[:])
            pend[0] = (ic, jb, ex, o, w)
            st["emitted"] += 1
            yield

    def norm_gen(ic):
        # chunk ic's last block is either still pending (flush it) or was
        # already flushed by a later-queued group's sc step
        if pend[0] is not None and pend[0][0] == ic:
            flush_pend()
        av = chunk_state[ic]["av"]
        # normalize + emit
        oun = pl.misc.tile([HS + 1, TI], F32, tag="oun")
        nc.vector.tensor_copy(out=oun[:], in_=av[:])
        yield
        otp = pl.ps_sc.tile([P, TI], F32, tag="sc")
        for tt in range(GG):
            nc.tensor.transpose(
                otp[:, tt * (HS + 1) : (tt + 1) * (HS + 1)],
                oun[:, tt * P : (tt + 1) * P],
                ident[0 : HS + 1, 0 : HS + 1],
            )
        yield
        rden = pl.misc.tile([P, GG], F32, tag="rden")
        for tt in range(GG):
            nc.vector.reciprocal(
                out=rden[:, tt : tt + 1],
                in_=otp[:, tt * (HS + 1) + HS : (tt + 1) * (HS + 1)],
            )
        osb = pl.misc.tile([P, GG, HS], F32, tag="osb")
        for tt in range(GG):
            nc.vector.tensor_scalar_mul(
                out=osb[:, tt, :],
                in0=otp[:, tt * (HS + 1) : tt * (HS + 1) + HS],
                scalar1=rden[:, tt : tt + 1],
            )
        nc.sync.dma_start(
            out=out[ic * TI : (ic + 1) * TI, :].rearrange(
                "(tt p) h -> p tt h", p=P
            ),
            in_=osb[:],
        )
        yield

    from collections import deque

    fore_q = deque()
    norms_q = deque()  # (ic, gen) — gated until the chunk is fully emitted

    def step_norm():
        while norms_q:
            ic, gen = norms_q[0]
            st = chunk_state.get(ic)
            if st is None or st["emitted"] < st["total"]:
                return False
            try:
                next(gen)
                return True
            except StopIteration:
                norms_q.popleft()
        return False

    def pump():
        """Run one ready attention step (norms preferred once safe)."""
        if step_norm():
            return True
        while fore_q:
            try:
                next(fore_q[0])
                return True
            except StopIteration:
                fore_q.popleft()
        return False

    def run_tp(name, g):
        for _ in tp_gen(name, g):
            pump()

    # q load-groups descend (q3 first) so the last-arriving loads feed only
    # the smallest remaining attention work (tail = groups (0,0) and (3,3))
    qorder = [NG - 1 - s for s in range(NG)]
    if not skip_g0_loads:
        for name, x, g0 in (("q", q, qorder[0]), ("k", k, 0), ("v", v, 0)):
            emit_load(nc, pl, name, x, g0)
    for s in range(NG):
        if s + 1 < NG:
            for name, x, gn in (("q", q, qorder[s + 1]), ("k", k, s + 1), ("v", v, s + 1)):
                emit_load(nc, pl, name, x, gn)
        def add_norm(ic):
            norms_q.append((ic, norm_gen(ic)))

        run_tp("q", qorder[s])
        if phase >= 4:
            # groups enabled by this stage's q projection (k/v from earlier)
            qg = qorder[s]
            hi = min(qg, s - 1)
            if hi >= 0:
                for jbg in range(hi, -1, -1):  # diag-first
                    fore_q.append(attn_group_gen(qg, jbg))
                if hi == qg:
                    add_norm(qg)
        run_tp("k", s)
        run_tp("v", s)
        if phase >= 4:
            # groups enabled by this stage's k/v projections (jbg = s)
            for ic in range(NG - 1, max(s, NG - 1 - s) - 1, -1):
                fore_q.append(attn_group_gen(ic, s))
                if s == ic:
                    add_norm(ic)

    # drain: round-robin the leftover block groups so the tail chunks
    # pipeline across engines; norms fire as their chunks complete
    while fore_q or norms_q:
        pn = step_norm()
        pb = False
        if fore_q:
            try:
                next(fore_q[0])
                pb = True
                if len(fore_q) > 1:
                    fore_q.rotate(-1)
            except StopIteration:
                fore_q.popleft()
                pb = True
        if not (pn or pb) and not fore_q:
            assert not norms_q, "norm queue stuck with incomplete chunk"

    if phase < 4:
        dummy = pl.misc.tile([P, HS], F32, tag="dummy")
        if phase < 2:
            nc.vector.tensor_copy(out=dummy[:], in_=last[0][:, 0, 0:HS])
        elif phase < 3:
            nc.vector.tensor_copy(out=dummy[:], in_=last[0][:, 0, 0, 0:HS])
        else:
            nc.vector.tensor_copy(out=dummy[:], in_=vx[:, 0, 0:HS])
        nc.sync.dma_start(out=out[0:P, :], in_=dummy[:])


def build_nc(n_iters: int = 1, phase: int = 4, split: bool = True):
    from contextlib import ExitStack

    # 64KB SWDGE ring (4096 descriptors) so up to 8 in-flight 512-descriptor
    # cast loads never block the Pool engine on descriptor-ring space.
    nc = bass.Bass(
        trn_type="TRN2", num_devices=B, dynamic_dma_scratch_size=65536
    )
    q = nc.declare_dram_parameter("q_vec", [T, C], F32, isOutput=False)
    k = nc.declare_dram_parameter("k_vec", [T, C], F32, isOutput=False)
    v = nc.declare_dram_parameter("v_vec", [T, C], F32, isOutput=False)
    mask = nc.declare_dram_parameter("mask", [T], I32, isOutput=False)
    wq = nc.declare_dram_parameter("Wq", [C, HS], F32, isOutput=False)
    wk = nc.declare_dram_parameter("Wk", [C, HS], F32, isOutput=False)
    wv = nc.declare_dram_parameter("Wv", [C, HS], F32, isOutput=False)
    out = nc.declare_dram_parameter("out", [T, HS], F32, isOutput=True)

    with tile.TileContext(nc) as tc:
        with ExitStack() as ctx:
            singles = ctx.enter_context(tc.tile_pool(name="singles", bufs=1))
            pl = Pools(tc, ctx)
            # the first data loads go ahead of the (HWDGE-issued) consts so
            # the Pool engine starts SWDGE descriptor generation immediately
            # (q loads descend: group NG-1 first — see attention_body).
            # With split=True the very first load lands per-tile so the PE
            # starts on the first 0.5MB instead of the first 2MB.
            if split:
                emit_load_split(nc, pl, "q", q.ap(), NG - 1)
            else:
                emit_load(nc, pl, "q", q.ap(), NG - 1)
            for name, x, g0 in (("k", k, 0), ("v", v, 0)):
                emit_load(nc, pl, name, x.ap(), g0)
            consts = make_consts(tc, singles, mask.ap(), wq.ap(), wk.ap(), wv.ap())
            for it in range(n_iters):
                attention_body(
                    tc,
                    consts,
                    pl,
                    q.ap(),
                    k.ap(),
                    v.ap(),
                    out.ap(),
                    phase=phase,
                    skip_g0_loads=(it == 0),
                )

    split_excess_waits(nc)
    return nc


# ---------------------------------------------------------------------------
# SPMD runner (compile once, execute via PJRT on the 8 axon cores)
# ---------------------------------------------------------------------------
class _Runner:
    def __init__(self, nc, n_cores=B):
        import jax
        from jax.sharding import Mesh, PartitionSpec
        from jax.experimental.shard_map import shard_map
        from concourse.bass2jax import (
            _bass_exec_p,
            install_neuronx_cc_hook,
            partition_id_tensor,
        )

        install_neuronx_cc_hook()
        self.jax = jax
        self.n_cores = n_cores
        partition_name = (
            nc.partition_id_tensor.name if nc.partition_id_tensor else None
        )

        in_names, out_names, out_avals, zero_outs = [], [], [], []
        for alloc in nc.m.functions[0].allocations:
            if not isinstance(alloc, mybir.MemoryLocationSet):
                continue
            name = alloc.memorylocations[0].name
            if alloc.kind == "ExternalInput":
                if name != partition_name:
                    in_names.append(name)
            elif alloc.kind == "ExternalOutput":
                out_names.append(name)
                shape = tuple(alloc.tensor_shape)
                dtype = mybir.dt.np(alloc.dtype)
                out_avals.append(jax.core.ShapedArray(shape, dtype))
                zero_outs.append(np.zeros(shape, dtype))
        self.in_names = list(in_names)
        self.out_names = out_names
        self.out_avals = out_avals
        self.zero_outs = zero_outs
        n_params = len(in_names)
        self.n_params = n_params

        all_in_names = list(in_names) + list(out_names)
        if partition_name is not None:
            all_in_names.append(partition_name)

        def _body(*args):
            operands = list(args)
            if partition_name is not None:
                operands.append(partition_id_tensor())
            outs = _bass_exec_p.bind(
                *operands,
                out_avals=tuple(out_avals),
                in_names=tuple(all_in_names),
                out_names=tuple(out_names),
                lowering_input_output_aliases=(),
                sim_require_finite=True,
                sim_require_nnan=True,
                nc=nc,
            )
            return tuple(outs)

        devices = jax.devices()[:n_cores]
        mesh = Mesh(np.asarray(devices), ("core",))
        n_outs = len(out_names)
        self.fn = jax.jit(
            shard_map(
                _body,
                mesh=mesh,
                in_specs=(PartitionSpec("core"),) * (n_params + n_outs),
                out_specs=(PartitionSpec("core"),) * n_outs,
                check_rep=False,
            ),
            keep_unused=True,
        )

    def prepare(self, in_maps):
        n = self.n_cores
        per_core = [[np.asarray(m[nm]) for nm in self.in_names] for m in in_maps]
        concat_in = [
            np.concatenate([per_core[c][i] for c in range(n)], axis=0)
            for i in range(self.n_params)
        ]
        concat_zeros = [
            np.zeros((n * z.shape[0], *z.shape[1:]), z.dtype) for z in self.zero_outs
        ]
        self.args = [self.jax.device_put(a) for a in concat_in + concat_zeros]
        return self

    def run(self):
        outs = self.fn(*self.args)
        self.jax.block_until_ready(outs)
        return outs

    def results(self, outs):
        n = self.n_cores
        return [
            {
                nm: np.asarray(outs[i]).reshape(n, *self.out_avals[i].shape)[c]
                for i, nm in enumerate(self.out_names)
            }
            for c in range(n)
        ]


_CACHED = {}


def _get_runner(n_iters: int = 1, phase: int = 4, split: bool = True):
    key = (n_iters, phase, split)
    if key not in _CACHED:
        _CACHED[key] = _Runner(build_nc(n_iters, phase, split))
    return _CACHED[key]


def kernel(q_vec, k_vec, v_vec, mask, Wq, Wk, Wv):
    q_vec = np.ascontiguousarray(np.asarray(q_vec, dtype=np.float32))
    k_vec = np.ascontiguousarray(np.asarray(k_vec, dtype=np.float32))
    v_vec = np.ascontiguousarray(np.asarray(v_vec, dtype=np.float32))
    mask = np.ascontiguousarray(np.asarray(mask, dtype=np.int32))
    Wq = np.ascontiguousarray(np.asarray(Wq, dtype=np.float32))
    Wk = np.ascontiguousarray(np.asarray(Wk, dtype=np.float32))
    Wv = np.ascontiguousarray(np.asarray(Wv, dtype=np.float32))

    r = _get_runner()
    in_maps = [
        {
            "q_vec": q_vec[b],
            "k_vec": k_vec[b],
            "v_vec": v_vec[b],
            "mask": mask[b],
            "Wq": Wq,
            "Wk": Wk,
            "Wv": Wv,
        }
        for b in range(B)
    ]
    r.prepare(in_maps)
    res = r.results(r.run())
    return np.stack([res[b]["out"] for b in range(B)], axis=0)
